# revision 8
# baseline (speedup 1.0000x reference)
"""Trainium2 Bass kernel v3 for nn_CRModule (retrieval_knn).

scores[i,j] = (wa[i]+wb[j])^2 * max(na[i]+nb[j] - 2 a_i.b_j, 0)

Two SPMD launches, ZERO device collectives (the CC engine costs ~70us
of serial init+op time per launch; a host combine of 64KB of partials
is free by comparison):

Launch A (DMA-bound ~46us): fc row-share [1536, 4096] bf16 (cols
  pre-split [even|odd]) -> in-place wide DVE folds -> gpsimd
  partition_all_reduce -> wsum partial [1,4096]. Packed x t-share
  [xa|xb] fp8 -> scalar squares -> folds -> [na|nb] partial [1,4096].
Host: sums the 8 partials (numpy, 64KB).
Launch B (PE-bound ~38us): 4x2 grid, core (r=d//2, c=d%2) owns
  scores[512r:+512, 1024c:+1024]. Packed fp8 k-groups [xa*-2|xb],
  k-outer DoubleRow matmul into all 8 psum banks; epilogue vectors
  (nav/wav per-partition, wb/nb rows) host-sliced; fused psum epilogue;
  bf16 store, host upcast.
"""

import numpy as np
import ml_dtypes

import concourse.bass as bass
import concourse.tile as tile
from concourse import bacc, mybir
from concourse.bass_isa import ReduceOp
from concourse.bass_utils import run_bass_kernel_spmd

BF16 = mybir.dt.bfloat16
F32 = mybir.dt.float32
FP8 = mybir.dt.float8e4
NP_BF16 = ml_dtypes.bfloat16
NP_FP8 = ml_dtypes.float8_e4m3

D = 8
T = 4096
KT = 32
C = 4096
CA = 2048
O = 12288
MR = 512          # rows per core in launch B
NC = 1024         # cols per core in launch B
MT = MR // 128    # 4
NJ = NC // 512    # 2
XG = 4            # k-tiles per packed x group
NXG = KT // XG    # 8
FRT = 12          # fc row-tiles per core in launch A (1536 rows)
F_CH = 4          # fc dma chunks
FRC = FRT // F_CH

_cache = {}


def _tree_fold(nc, src, n):
    """In-place pairwise tree over src[:, i, :]; result in src[:, 0, :]."""
    stride = 1
    while stride < n:
        for i in range(0, n - stride, 2 * stride):
            nc.vector.tensor_add(src[:, i, :], src[:, i, :],
                                 src[:, i + stride, :])
        stride *= 2


def _build_a():
    nc = bacc.Bacc("TRN2", target_bir_lowering=False, debug=False, num_devices=D)
    fcr_d = nc.dram_tensor("fcr", [128, FRT, C], BF16, kind="ExternalInput").ap()
    xsq_d = nc.dram_tensor("xsq", [128, XG, C], FP8, kind="ExternalInput").ap()
    w_d = nc.dram_tensor("wpart", [1, C], F32, kind="ExternalOutput").ap()
    n_d = nc.dram_tensor("npart", [1, C], F32, kind="ExternalOutput").ap()

    CH = (3, 3, 3, 2, 1)  # fc row-tiles per chunk; small last -> short tail

    with tile.TileContext(nc) as tc:
        with (
            tc.tile_pool(name="fcp", bufs=3) as fcp,
            tc.tile_pool(name="xp", bufs=1) as xp,
            tc.tile_pool(name="sq", bufs=1) as sqp,
            tc.tile_pool(name="small", bufs=1) as small,
        ):
            ones = small.tile([128, 1], BF16)
            nc.vector.memset(ones[:], 1.0)
            nst = small.tile([1, C], F32)
            wst = small.tile([1, C], F32)

            # x rides the gpsimd (SWDGE) ring so fc owns the sync ring
            # head; squares + k-fold on DVE while fc streams
            x_t = xp.tile([128, XG, C], FP8)
            nc.gpsimd.dma_start(x_t[:], xsq_d[:])
            fck = []
            o0 = 0
            for j, n in enumerate(CH):
                f = fcp.tile([128, n, C], BF16, tag=f"fc{j}", bufs=1)
                nc.sync.dma_start(f[:], fcr_d[:, o0:o0 + n, :])
                fck.append(f)
                o0 += n
            # squares on DVE (fast elementwise engine: ~2.3us per k-tile;
            # scalar is 7us, gpsimd 14us for the same op), then fold the
            # k-tiles so the post-fc nn matmul tail is 8 MMs instead of 32
            s = sqp.tile([128, XG, C], BF16)
            for h in range(XG):
                nc.vector.tensor_mul(s[:, h, :], x_t[:, h, :], x_t[:, h, :])
            _tree_fold(nc, s, XG)

            # fc column sums FIRST on the PE (chase the arriving chunks);
            # ones-matmuls accumulate into 8 psum banks, no DVE folds
            with tc.tile_pool(name="psw", bufs=1, space="PSUM") as psw:
                pw = [psw.tile([1, 512], F32, name=f"pw{cc}", tag=f"pw{cc}")
                      for cc in range(C // 512)]
                ot_idx = 0
                for j, n in enumerate(CH):
                    for k in range(n):
                        for cc in range(C // 512):
                            nc.tensor.matmul(
                                pw[cc][:], ones[:],
                                fck[j][:, k, cc * 512:(cc + 1) * 512],
                                start=(ot_idx == 0), stop=(ot_idx == FRT - 1))
                        ot_idx += 1
                for cc in range(C // 512):
                    nc.vector.tensor_copy(wst[0:1, cc * 512:(cc + 1) * 512],
                                          pw[cc][:])
            nc.sync.dma_start(w_d[:], wst[:])

            with tc.tile_pool(name="psn", bufs=1, space="PSUM") as psn:
                for cc in range(C // 512):
                    pt = psn.tile([1, 512], F32, name=f"pn{cc}", tag=f"pn{cc}")
                    nc.tensor.matmul(
                        pt[:], ones[:], s[:, 0, cc * 512:(cc + 1) * 512],
                        start=True, stop=True)
                    nc.vector.tensor_copy(nst[0:1, cc * 512:(cc + 1) * 512],
                                          pt[:])
            nc.sync.dma_start(n_d[:], nst[:])

    nc.compile()
    return nc


def _build_b():
    nc = bacc.Bacc("TRN2", target_bir_lowering=False, debug=False, num_devices=D)
    xpk_d = nc.dram_tensor("xpk", [128, KT, 1536], FP8, kind="ExternalInput").ap()
    wav_d = nc.dram_tensor("wav", [128, MT], F32, kind="ExternalInput").ap()
    # [0,0:NC]=wb_own, [0,NC:2NC]=nb_own, [0,2NC:2NC+MR]=na_own
    fv_d = nc.dram_tensor("fv", [1, 2 * NC + MR], F32, kind="ExternalInput").ap()
    out_d = nc.dram_tensor("scores", [MR, NC], BF16, kind="ExternalOutput").ap()

    with tile.TileContext(nc) as tc:
        with (
            tc.tile_pool(name="xp", bufs=1) as xp,
            tc.tile_pool(name="small", bufs=1) as small,
            tc.tile_pool(name="epi", bufs=1) as epip,
            tc.tile_pool(name="outp", bufs=2) as outp,
            tc.tile_pool(name="psm", bufs=1, space="PSUM") as psm,
        ):
            # x groups get the sync ring immediately (4x 1.57MB: fewer
            # per-DMA gaps); epilogue vectors ride the gpsimd ring
            BXG, BN = 8, 4
            xg = []
            for i in range(BN):
                x_t = xp.tile([128, BXG, 1536], FP8, tag=f"xg{i}")
                nc.sync.dma_start(x_t[:], xpk_d[:, i * BXG:(i + 1) * BXG, :])
                xg.append(x_t)
            wav = small.tile([128, MT], F32)
            nc.gpsimd.dma_start(wav[:], wav_d[:])
            wb_bc = epip.tile([128, NC], F32)
            nc.gpsimd.dma_start(wb_bc[:],
                                fv_d[0:1, 0:NC].to_broadcast([128, NC]))
            vrow = small.tile([1, NC + MR], F32)
            nc.gpsimd.dma_start(vrow[:], fv_d[0:1, NC:2 * NC + MR])
            nbna = small.tile([1, NC + MR], BF16)
            nc.vector.tensor_copy(nbna[:], vrow[:])
            ones_c = small.tile([1, 128], BF16)
            nc.vector.memset(ones_c[:], 1.0)
            ones_r = small.tile([1, 512], BF16)
            nc.vector.memset(ones_r[:], 1.0)
            # PE warmup: keep HAM busy from t~1us so the real matmul stream
            # runs at 2.4GHz from its first instruction
            wrm = small.tile([128, 2, 640], FP8)
            nc.vector.memset(wrm[:], 0.0)

            # w2[m] = (wa[i] + wb[j])^2 on ScalarE (overlaps x DMA)
            w2 = []
            for m in range(MT):
                w2m = epip.tile([128, NC], F32, name=f"w2_{m}", tag=f"w2{m}")
                nc.scalar.activation(
                    w2m[:], wb_bc[:], mybir.ActivationFunctionType.Square,
                    bias=wav[:, m:m + 1], scale=1.0)
                w2.append(w2m)

            ps = [psm.tile([128, NJ, 512], F32, name=f"ps{m}", tag=f"ps{m}")
                  for m in range(MT)]
            for _ in range(12):
                nc.tensor.matmul(
                    ps[0][:, 0, :], wrm[:, :, 0:128], wrm[:, :, 128:640],
                    start=True, stop=True, skip_group_check=True,
                    perf_mode=mybir.MatmulPerfMode.DoubleRow)
            # hybrid order: k-outer over chunks 0..2 (PE chases arrivals at
            # full density), then chunk 3 per-m with staggered stop so each
            # m-tile's epilogue overlaps the remaining matmuls
            for i in range(BN - 1):
                for s2 in range(BXG // 2):
                    first = (i == 0 and s2 == 0)
                    for m in range(MT):
                        for nj in range(NJ):
                            nc.tensor.matmul(
                                ps[m][:, nj, :],
                                xg[i][:, 2 * s2:2 * s2 + 2,
                                      m * 128:(m + 1) * 128],
                                xg[i][:, 2 * s2:2 * s2 + 2,
                                      MR + nj * 512:MR + (nj + 1) * 512],
                                start=first, stop=False,
                                perf_mode=mybir.MatmulPerfMode.DoubleRow)
            for m in range(MT):
                for s2 in range(BXG // 2):
                    for nj in range(NJ):
                        nc.tensor.matmul(
                            ps[m][:, nj, :],
                            xg[BN - 1][:, 2 * s2:2 * s2 + 2,
                                       m * 128:(m + 1) * 128],
                            xg[BN - 1][:, 2 * s2:2 * s2 + 2,
                                       MR + nj * 512:MR + (nj + 1) * 512],
                            start=False, stop=False,
                            perf_mode=mybir.MatmulPerfMode.DoubleRow)
                # fold na[i] / nb[j] into psum via K=1 matmuls
                for nj in range(NJ):
                    nc.tensor.matmul(
                        ps[m][:, nj, :],
                        nbna[0:1, NC + m * 128:NC + (m + 1) * 128],
                        ones_r[:],
                        start=False, stop=False, skip_group_check=True)
                    nc.tensor.matmul(
                        ps[m][:, nj, :],
                        ones_c[0:1, 0:128],
                        nbna[0:1, nj * 512:(nj + 1) * 512],
                        start=False, stop=True, skip_group_check=True)
                # epilogue: Scalar relu (psum->sbuf), DVE mul by w2, store
                pflat = ps[m].rearrange("p a b -> p (a b)")
                rl = outp.tile([128, NC], F32, tag="rl")
                nc.scalar.activation(rl[:], pflat,
                                     mybir.ActivationFunctionType.Relu)
                ot = outp.tile([128, NC], BF16, tag="ot")
                nc.vector.tensor_mul(ot[:], rl[:], w2[m][:])
                nc.sync.dma_start(out_d[m * 128:(m + 1) * 128, :], ot[:])

    nc.compile()
    return nc


def _p_major(a, np_dtype):
    n = a.shape[0] // 128
    return np.ascontiguousarray(
        a.reshape(n, 128, a.shape[1]).transpose(1, 0, 2).astype(np_dtype))


def kernel_v3(x, fc_weight, _trace=False):
    x = np.asarray(x, dtype=np.float32)
    fc = np.asarray(fc_weight, dtype=np.float32)
    xf = x.reshape(T, C)
    xa = np.ascontiguousarray(xf[:, 0::2])
    xb = np.ascontiguousarray(xf[:, 1::2])
    fc_r = np.concatenate([fc[:, 0::2], fc[:, 1::2]], axis=1)  # [O, even|odd]

    if "a" not in _cache:
        _cache["a"] = _build_a()
    if "b" not in _cache:
        _cache["b"] = _build_b()

    # ---- launch A ----
    in_a = []
    for d in range(D):
        xs = np.concatenate(
            [xa[512 * d:512 * (d + 1), :], xb[512 * d:512 * (d + 1), :]],
            axis=1)                                   # [512, 4096] = [xa|xb]
        in_a.append({
            "fcr": _p_major(fc_r[1536 * d:1536 * (d + 1), :], NP_BF16),
            "xsq": _p_major(xs, NP_FP8),
        })
    res_a = run_bass_kernel_spmd(_cache["a"], in_a, core_ids=list(range(D)),
                                 trace=_trace)
    t_a = res_a.exec_time_ns

    wsum = np.sum([res_a.results[d]["wpart"][0] for d in range(D)], axis=0,
                  dtype=np.float32)                   # [4096] = [wa|wb]
    nn = np.sum([res_a.results[d]["npart"][0] for d in range(D)], axis=0,
                dtype=np.float32)                     # [4096] = [na|nb]
    wa, wb = wsum[:CA], wsum[CA:]
    na, nb = nn[:CA], nn[CA:]

    # ---- launch B ----
    xa2 = xa * -2.0
    in_b = []
    pidx = (np.arange(128)[:, None] + 128 * np.arange(MT)[None, :])  # [128,MT]
    for d in range(D):
        r, c = d // 2, d % 2
        xa_p = _p_major(xa2[:, 512 * r:512 * (r + 1)], NP_FP8)
        xb_p = _p_major(xb[:, NC * c:NC * (c + 1)], NP_FP8)
        xpk = np.ascontiguousarray(np.concatenate([xa_p, xb_p], axis=2))
        in_b.append({
            "xpk": xpk,
            "wav": np.ascontiguousarray(wa[512 * r + pidx]).astype(np.float32),
            "fv": np.concatenate([wb[NC * c:NC * (c + 1)],
                                  nb[NC * c:NC * (c + 1)],
                                  na[512 * r:512 * (r + 1)]]).reshape(1, -1)
                    .astype(np.float32),
        })
    res_b = run_bass_kernel_spmd(_cache["b"], in_b, core_ids=list(range(D)),
                                 trace=_trace)
    t_b = res_b.exec_time_ns

    out = np.empty((CA, CA), dtype=np.float32)
    for d in range(D):
        r, c = d // 2, d % 2
        out[512 * r:512 * (r + 1), NC * c:NC * (c + 1)] = (
            res_b.results[d]["scores"].astype(np.float32))
    if _trace:
        kernel_v3.last_times = (t_a, t_b)
    return out


def kernel(x, fc_weight):
    """Graded entrypoint: full inputs in, full [2048, 2048] scores out."""
    return kernel_v3(x, fc_weight)


# revision 9
# speedup vs baseline: 1.0201x; 1.0201x over previous
"""Trainium2 Bass kernel v3 for nn_CRModule (retrieval_knn).

scores[i,j] = (wa[i]+wb[j])^2 * max(na[i]+nb[j] - 2 a_i.b_j, 0)

Two SPMD launches, ZERO device collectives (the CC engine costs ~70us
of serial init+op time per launch; a host combine of 64KB of partials
is free by comparison):

Launch A (DMA-bound ~46us): fc row-share [1536, 4096] bf16 (cols
  pre-split [even|odd]) -> in-place wide DVE folds -> gpsimd
  partition_all_reduce -> wsum partial [1,4096]. Packed x t-share
  [xa|xb] fp8 -> scalar squares -> folds -> [na|nb] partial [1,4096].
Host: sums the 8 partials (numpy, 64KB).
Launch B (PE-bound ~38us): 4x2 grid, core (r=d//2, c=d%2) owns
  scores[512r:+512, 1024c:+1024]. Packed fp8 k-groups [xa*-2|xb],
  k-outer DoubleRow matmul into all 8 psum banks; epilogue vectors
  (nav/wav per-partition, wb/nb rows) host-sliced; fused psum epilogue;
  bf16 store, host upcast.
"""

import numpy as np
import ml_dtypes

import concourse.bass as bass
import concourse.tile as tile
from concourse import bacc, mybir
from concourse.bass_isa import ReduceOp
from concourse.bass_utils import run_bass_kernel_spmd

BF16 = mybir.dt.bfloat16
F32 = mybir.dt.float32
FP8 = mybir.dt.float8e4
NP_BF16 = ml_dtypes.bfloat16
NP_FP8 = ml_dtypes.float8_e4m3

D = 8
T = 4096
KT = 32
C = 4096
CA = 2048
O = 12288
MR = 512          # rows per core in launch B
NC = 1024         # cols per core in launch B
MT = MR // 128    # 4
NJ = NC // 512    # 2
XG = 4            # k-tiles per packed x group
NXG = KT // XG    # 8
FRT = 12          # fc row-tiles per core in launch A (1536 rows)
F_CH = 4          # fc dma chunks
FRC = FRT // F_CH

_cache = {}


def _tree_fold(nc, src, n):
    """In-place pairwise tree over src[:, i, :]; result in src[:, 0, :]."""
    stride = 1
    while stride < n:
        for i in range(0, n - stride, 2 * stride):
            nc.vector.tensor_add(src[:, i, :], src[:, i, :],
                                 src[:, i + stride, :])
        stride *= 2


def _build_a():
    nc = bacc.Bacc("TRN2", target_bir_lowering=False, debug=False, num_devices=D)
    fcr_d = nc.dram_tensor("fcr", [128, FRT, C], BF16, kind="ExternalInput").ap()
    xsq_d = nc.dram_tensor("xsq", [128, XG, C], FP8, kind="ExternalInput").ap()
    w_d = nc.dram_tensor("wpart", [1, C], F32, kind="ExternalOutput").ap()
    n_d = nc.dram_tensor("npart", [1, C], F32, kind="ExternalOutput").ap()

    CH = (3, 3, 3, 2, 1)  # fc row-tiles per chunk; small last -> short tail

    with tile.TileContext(nc) as tc:
        with (
            tc.tile_pool(name="fcp", bufs=3) as fcp,
            tc.tile_pool(name="xp", bufs=1) as xp,
            tc.tile_pool(name="sq", bufs=1) as sqp,
            tc.tile_pool(name="small", bufs=1) as small,
        ):
            ones = small.tile([128, 1], BF16)
            nc.vector.memset(ones[:], 1.0)
            nst = small.tile([1, C], F32)
            wst = small.tile([1, C], F32)

            # x first on the sync ring (lands ~13us); fc follows
            x_t = xp.tile([128, XG, C], FP8)
            nc.sync.dma_start(x_t[:], xsq_d[:])
            fck = []
            o0 = 0
            for j, n in enumerate(CH):
                f = fcp.tile([128, n, C], BF16, tag=f"fc{j}", bufs=1)
                nc.sync.dma_start(f[:], fcr_d[:, o0:o0 + n, :])
                fck.append(f)
                o0 += n
            # squares split scalar||DVE (each ~3.5-4.4us per k-tile on fp8
            # input), folds on DVE; nn ready ~29us
            s = sqp.tile([128, XG, C], BF16)
            nc.scalar.square(s[:, 1, :], x_t[:, 1, :])
            nc.scalar.square(s[:, 2, :], x_t[:, 2, :])
            nc.scalar.square(s[:, 3, :], x_t[:, 3, :])
            nc.vector.tensor_mul(s[:, 0, :], x_t[:, 0, :], x_t[:, 0, :])
            nc.vector.tensor_add(s[:, 0, :], s[:, 0, :], s[:, 1, :])
            nc.vector.tensor_add(s[:, 2, :], s[:, 2, :], s[:, 3, :])
            nc.vector.tensor_add(s[:, 0, :], s[:, 0, :], s[:, 2, :])

            # nn psum scope first: its 8 banks are used and freed before the
            # fc accumulation claims them (PE FIFO: nn MMs ~29-31us, the fc
            # chunks are still arriving until ~50)
            with tc.tile_pool(name="psn", bufs=1, space="PSUM") as psn:
                for cc in range(C // 512):
                    pt = psn.tile([1, 512], F32, name=f"pn{cc}", tag=f"pn{cc}")
                    nc.tensor.matmul(
                        pt[:], ones[:], s[:, 0, cc * 512:(cc + 1) * 512],
                        start=True, stop=True)
                    nc.vector.tensor_copy(nst[0:1, cc * 512:(cc + 1) * 512],
                                          pt[:])
            nc.sync.dma_start(n_d[:], nst[:])

            # fc column sums on the PE; final chunk stops per-column-chunk
            # so the psum->sbuf copies interleave with the matmul tail
            with tc.tile_pool(name="psw", bufs=1, space="PSUM") as psw:
                pw = [psw.tile([1, 512], F32, name=f"pw{cc}", tag=f"pw{cc}")
                      for cc in range(C // 512)]
                ot_idx = 0
                for j, n in enumerate(CH[:-1]):
                    for k in range(n):
                        for cc in range(C // 512):
                            nc.tensor.matmul(
                                pw[cc][:], ones[:],
                                fck[j][:, k, cc * 512:(cc + 1) * 512],
                                start=(ot_idx == 0), stop=False)
                        ot_idx += 1
                for cc in range(C // 512):
                    nc.tensor.matmul(
                        pw[cc][:], ones[:],
                        fck[-1][:, 0, cc * 512:(cc + 1) * 512],
                        start=False, stop=True)
                    nc.vector.tensor_copy(wst[0:1, cc * 512:(cc + 1) * 512],
                                          pw[cc][:])
            nc.sync.dma_start(w_d[:], wst[:])

    nc.compile()
    return nc


def _build_b():
    nc = bacc.Bacc("TRN2", target_bir_lowering=False, debug=False, num_devices=D)
    xpk_d = nc.dram_tensor("xpk", [128, KT, 1536], FP8, kind="ExternalInput").ap()
    wav_d = nc.dram_tensor("wav", [128, MT], F32, kind="ExternalInput").ap()
    # [0,0:NC]=wb_own, [0,NC:2NC]=nb_own, [0,2NC:2NC+MR]=na_own
    fv_d = nc.dram_tensor("fv", [1, 2 * NC + MR], F32, kind="ExternalInput").ap()
    out_d = nc.dram_tensor("scores", [MR, NC], BF16, kind="ExternalOutput").ap()

    with tile.TileContext(nc) as tc:
        with (
            tc.tile_pool(name="xp", bufs=1) as xp,
            tc.tile_pool(name="small", bufs=1) as small,
            tc.tile_pool(name="epi", bufs=1) as epip,
            tc.tile_pool(name="outp", bufs=2) as outp,
            tc.tile_pool(name="psm", bufs=1, space="PSUM") as psm,
        ):
            # x groups get the sync ring immediately (4x 1.57MB: fewer
            # per-DMA gaps); epilogue vectors ride the gpsimd ring
            BXG, BN = 8, 4
            xg = []
            for i in range(BN):
                x_t = xp.tile([128, BXG, 1536], FP8, tag=f"xg{i}")
                nc.sync.dma_start(x_t[:], xpk_d[:, i * BXG:(i + 1) * BXG, :])
                xg.append(x_t)
            wav = small.tile([128, MT], F32)
            nc.gpsimd.dma_start(wav[:], wav_d[:])
            wb_bc = epip.tile([128, NC], F32)
            nc.gpsimd.dma_start(wb_bc[:],
                                fv_d[0:1, 0:NC].to_broadcast([128, NC]))
            vrow = small.tile([1, NC + MR], F32)
            nc.gpsimd.dma_start(vrow[:], fv_d[0:1, NC:2 * NC + MR])
            nbna = small.tile([1, NC + MR], BF16)
            nc.vector.tensor_copy(nbna[:], vrow[:])
            ones_c = small.tile([1, 128], BF16)
            nc.vector.memset(ones_c[:], 1.0)
            ones_r = small.tile([1, 512], BF16)
            nc.vector.memset(ones_r[:], 1.0)
            # PE warmup: keep HAM busy from t~1us so the real matmul stream
            # runs at 2.4GHz from its first instruction
            wrm = small.tile([128, 2, 640], FP8)
            nc.vector.memset(wrm[:], 0.0)

            # w2[m] = (wa[i] + wb[j])^2 on ScalarE (overlaps x DMA)
            w2 = []
            for m in range(MT):
                w2m = epip.tile([128, NC], F32, name=f"w2_{m}", tag=f"w2{m}")
                nc.scalar.activation(
                    w2m[:], wb_bc[:], mybir.ActivationFunctionType.Square,
                    bias=wav[:, m:m + 1], scale=1.0)
                w2.append(w2m)

            ps = [psm.tile([128, NJ, 512], F32, name=f"ps{m}", tag=f"ps{m}")
                  for m in range(MT)]
            for _ in range(12):
                nc.tensor.matmul(
                    ps[0][:, 0, :], wrm[:, :, 0:128], wrm[:, :, 128:640],
                    start=True, stop=True, skip_group_check=True,
                    perf_mode=mybir.MatmulPerfMode.DoubleRow)
            # hybrid order: k-outer over chunks 0..2 (PE chases arrivals at
            # full density), then chunk 3 per-m with staggered stop so each
            # m-tile's epilogue overlaps the remaining matmuls
            for i in range(BN - 1):
                for s2 in range(BXG // 2):
                    first = (i == 0 and s2 == 0)
                    for m in range(MT):
                        for nj in range(NJ):
                            nc.tensor.matmul(
                                ps[m][:, nj, :],
                                xg[i][:, 2 * s2:2 * s2 + 2,
                                      m * 128:(m + 1) * 128],
                                xg[i][:, 2 * s2:2 * s2 + 2,
                                      MR + nj * 512:MR + (nj + 1) * 512],
                                start=first, stop=False,
                                perf_mode=mybir.MatmulPerfMode.DoubleRow)
            for m in range(MT):
                for s2 in range(BXG // 2):
                    for nj in range(NJ):
                        nc.tensor.matmul(
                            ps[m][:, nj, :],
                            xg[BN - 1][:, 2 * s2:2 * s2 + 2,
                                       m * 128:(m + 1) * 128],
                            xg[BN - 1][:, 2 * s2:2 * s2 + 2,
                                       MR + nj * 512:MR + (nj + 1) * 512],
                            start=False, stop=False,
                            perf_mode=mybir.MatmulPerfMode.DoubleRow)
                # fold na[i] / nb[j] into psum via K=1 matmuls
                for nj in range(NJ):
                    nc.tensor.matmul(
                        ps[m][:, nj, :],
                        nbna[0:1, NC + m * 128:NC + (m + 1) * 128],
                        ones_r[:],
                        start=False, stop=False, skip_group_check=True)
                    nc.tensor.matmul(
                        ps[m][:, nj, :],
                        ones_c[0:1, 0:128],
                        nbna[0:1, nj * 512:(nj + 1) * 512],
                        start=False, stop=True, skip_group_check=True)
                # epilogue: Scalar relu (psum->sbuf), DVE mul by w2, store
                pflat = ps[m].rearrange("p a b -> p (a b)")
                rl = outp.tile([128, NC], F32, tag="rl")
                nc.scalar.activation(rl[:], pflat,
                                     mybir.ActivationFunctionType.Relu)
                ot = outp.tile([128, NC], BF16, tag="ot")
                nc.vector.tensor_mul(ot[:], rl[:], w2[m][:])
                nc.sync.dma_start(out_d[m * 128:(m + 1) * 128, :], ot[:])

    nc.compile()
    return nc


def _p_major(a, np_dtype):
    n = a.shape[0] // 128
    return np.ascontiguousarray(
        a.reshape(n, 128, a.shape[1]).transpose(1, 0, 2).astype(np_dtype))


def kernel_v3(x, fc_weight, _trace=False):
    x = np.asarray(x, dtype=np.float32)
    fc = np.asarray(fc_weight, dtype=np.float32)
    xf = x.reshape(T, C)
    xa = np.ascontiguousarray(xf[:, 0::2])
    xb = np.ascontiguousarray(xf[:, 1::2])
    fc_r = np.concatenate([fc[:, 0::2], fc[:, 1::2]], axis=1)  # [O, even|odd]

    if "a" not in _cache:
        _cache["a"] = _build_a()
    if "b" not in _cache:
        _cache["b"] = _build_b()

    # ---- launch A ----
    in_a = []
    for d in range(D):
        xs = np.concatenate(
            [xa[512 * d:512 * (d + 1), :], xb[512 * d:512 * (d + 1), :]],
            axis=1)                                   # [512, 4096] = [xa|xb]
        in_a.append({
            "fcr": _p_major(fc_r[1536 * d:1536 * (d + 1), :], NP_BF16),
            "xsq": _p_major(xs, NP_FP8),
        })
    res_a = run_bass_kernel_spmd(_cache["a"], in_a, core_ids=list(range(D)),
                                 trace=_trace)
    t_a = res_a.exec_time_ns

    wsum = np.sum([res_a.results[d]["wpart"][0] for d in range(D)], axis=0,
                  dtype=np.float32)                   # [4096] = [wa|wb]
    nn = np.sum([res_a.results[d]["npart"][0] for d in range(D)], axis=0,
                dtype=np.float32)                     # [4096] = [na|nb]
    wa, wb = wsum[:CA], wsum[CA:]
    na, nb = nn[:CA], nn[CA:]

    # ---- launch B ----
    xa2 = xa * -2.0
    in_b = []
    pidx = (np.arange(128)[:, None] + 128 * np.arange(MT)[None, :])  # [128,MT]
    for d in range(D):
        r, c = d // 2, d % 2
        xa_p = _p_major(xa2[:, 512 * r:512 * (r + 1)], NP_FP8)
        xb_p = _p_major(xb[:, NC * c:NC * (c + 1)], NP_FP8)
        xpk = np.ascontiguousarray(np.concatenate([xa_p, xb_p], axis=2))
        in_b.append({
            "xpk": xpk,
            "wav": np.ascontiguousarray(wa[512 * r + pidx]).astype(np.float32),
            "fv": np.concatenate([wb[NC * c:NC * (c + 1)],
                                  nb[NC * c:NC * (c + 1)],
                                  na[512 * r:512 * (r + 1)]]).reshape(1, -1)
                    .astype(np.float32),
        })
    res_b = run_bass_kernel_spmd(_cache["b"], in_b, core_ids=list(range(D)),
                                 trace=_trace)
    t_b = res_b.exec_time_ns

    out = np.empty((CA, CA), dtype=np.float32)
    for d in range(D):
        r, c = d // 2, d % 2
        out[512 * r:512 * (r + 1), NC * c:NC * (c + 1)] = (
            res_b.results[d]["scores"].astype(np.float32))
    if _trace:
        kernel_v3.last_times = (t_a, t_b)
    return out


def kernel(x, fc_weight):
    """Graded entrypoint: full inputs in, full [2048, 2048] scores out."""
    return kernel_v3(x, fc_weight)


# revision 10
# speedup vs baseline: 1.0625x; 1.0415x over previous
"""Trainium2 Bass kernel v3 for nn_CRModule (retrieval_knn).

scores[i,j] = (wa[i]+wb[j])^2 * max(na[i]+nb[j] - 2 a_i.b_j, 0)

Two SPMD launches, ZERO device collectives (the CC engine costs ~70us
of serial init+op time per launch; a host combine of 64KB of partials
is free by comparison):

Launch A (DMA-bound ~46us): fc row-share [1536, 4096] bf16 (cols
  pre-split [even|odd]) -> in-place wide DVE folds -> gpsimd
  partition_all_reduce -> wsum partial [1,4096]. Packed x t-share
  [xa|xb] fp8 -> scalar squares -> folds -> [na|nb] partial [1,4096].
Host: sums the 8 partials (numpy, 64KB).
Launch B (PE-bound ~38us): 4x2 grid, core (r=d//2, c=d%2) owns
  scores[512r:+512, 1024c:+1024]. Packed fp8 k-groups [xa*-2|xb],
  k-outer DoubleRow matmul into all 8 psum banks; epilogue vectors
  (nav/wav per-partition, wb/nb rows) host-sliced; fused psum epilogue;
  bf16 store, host upcast.
"""

import numpy as np
import ml_dtypes

import concourse.bass as bass
import concourse.tile as tile
from concourse import bacc, mybir
from concourse.bass_isa import ReduceOp
from concourse.bass_utils import run_bass_kernel_spmd

BF16 = mybir.dt.bfloat16
F32 = mybir.dt.float32
FP8 = mybir.dt.float8e4
NP_BF16 = ml_dtypes.bfloat16
NP_FP8 = ml_dtypes.float8_e4m3

D = 8
T = 4096
KT = 32
C = 4096
CA = 2048
O = 12288
MR = 512          # rows per core in launch B
NC = 1024         # cols per core in launch B
MT = MR // 128    # 4
NJ = NC // 512    # 2
XG = 4            # k-tiles per packed x group
NXG = KT // XG    # 8
FRT = 12          # fc row-tiles per core in launch A (1536 rows)
F_CH = 4          # fc dma chunks
FRC = FRT // F_CH

_cache = {}


def _tree_fold(nc, src, n):
    """In-place pairwise tree over src[:, i, :]; result in src[:, 0, :]."""
    stride = 1
    while stride < n:
        for i in range(0, n - stride, 2 * stride):
            nc.vector.tensor_add(src[:, i, :], src[:, i, :],
                                 src[:, i + stride, :])
        stride *= 2


def _build_a():
    nc = bacc.Bacc("TRN2", target_bir_lowering=False, debug=False, num_devices=D)
    fcr_d = nc.dram_tensor("fcr", [128, FRT, C], BF16, kind="ExternalInput").ap()
    xsq_d = nc.dram_tensor("xsq", [128, XG, C], FP8, kind="ExternalInput").ap()
    w_d = nc.dram_tensor("wpart", [1, C], F32, kind="ExternalOutput").ap()
    n_d = nc.dram_tensor("npart", [1, C], F32, kind="ExternalOutput").ap()

    CH = (3, 3, 3, 2, 1)  # fc row-tiles per chunk; small last -> short tail

    with tile.TileContext(nc) as tc:
        with (
            tc.tile_pool(name="fcp", bufs=3) as fcp,
            tc.tile_pool(name="xp", bufs=1) as xp,
            tc.tile_pool(name="sq", bufs=1) as sqp,
            tc.tile_pool(name="small", bufs=1) as small,
        ):
            ones = small.tile([128, 1], BF16)
            nc.vector.memset(ones[:], 1.0)
            nst = small.tile([1, C], F32)
            wst = small.tile([1, C], F32)

            # ring: fc0, fc1, x, fc2..4 — x lands ~31us, feeding the late
            # nn path; fc owns the ring head so w-matmuls start ~15us
            x_t = xp.tile([128, XG, C], FP8)
            fck = []
            o0 = 0
            for j, n in enumerate(CH):
                f = fcp.tile([128, n, C], BF16, tag=f"fc{j}", bufs=1)
                nc.sync.dma_start(f[:], fcr_d[:, o0:o0 + n, :])
                fck.append(f)
                o0 += n
                if j == 1:
                    nc.sync.dma_start(x_t[:], xsq_d[:])
            # squares split DVE||scalar, folds on DVE; nn ready ~45us
            s = sqp.tile([128, XG, C], BF16)
            nc.scalar.square(s[:, 1, :], x_t[:, 1, :])
            nc.scalar.square(s[:, 2, :], x_t[:, 2, :])
            nc.vector.tensor_mul(s[:, 0, :], x_t[:, 0, :], x_t[:, 0, :])
            nc.vector.tensor_mul(s[:, 3, :], x_t[:, 3, :], x_t[:, 3, :])
            nc.vector.tensor_add(s[:, 0, :], s[:, 0, :], s[:, 1, :])
            nc.vector.tensor_add(s[:, 2, :], s[:, 2, :], s[:, 3, :])
            nc.vector.tensor_add(s[:, 0, :], s[:, 0, :], s[:, 2, :])

            wjunk = small.tile([128, 512], BF16)
            nc.vector.memset(wjunk[:], 0.0)

            # fc column sums on the PE; final chunk stops per-column-chunk
            # so the psum->sbuf copies interleave with the matmul tail
            with tc.tile_pool(name="psw", bufs=1, space="PSUM") as psw:
                pw = [psw.tile([1, 512], F32, name=f"pw{cc}", tag=f"pw{cc}")
                      for cc in range(C // 512)]
                # HAM warmup: keep the PE busy until fc0 lands (~15us) so
                # the accumulation stream runs at 2.4GHz
                for _ in range(48):
                    nc.tensor.matmul(pw[0][:], ones[:], wjunk[:],
                                     start=True, stop=True,
                                     skip_group_check=True)
                ot_idx = 0
                for j, n in enumerate(CH[:-1]):
                    for k in range(n):
                        for cc in range(C // 512):
                            nc.tensor.matmul(
                                pw[cc][:], ones[:],
                                fck[j][:, k, cc * 512:(cc + 1) * 512],
                                start=(ot_idx == 0), stop=False)
                        ot_idx += 1
                for cc in range(C // 512):
                    nc.tensor.matmul(
                        pw[cc][:], ones[:],
                        fck[-1][:, 0, cc * 512:(cc + 1) * 512],
                        start=False, stop=True)
                    nc.vector.tensor_copy(wst[0:1, cc * 512:(cc + 1) * 512],
                                          pw[cc][:])
            nc.sync.dma_start(w_d[:], wst[:])

            # nn psum scope second: banks freed by the wst copies
            with tc.tile_pool(name="psn", bufs=1, space="PSUM") as psn:
                for cc in range(C // 512):
                    pt = psn.tile([1, 512], F32, name=f"pn{cc}", tag=f"pn{cc}")
                    nc.tensor.matmul(
                        pt[:], ones[:], s[:, 0, cc * 512:(cc + 1) * 512],
                        start=True, stop=True)
                    nc.vector.tensor_copy(nst[0:1, cc * 512:(cc + 1) * 512],
                                          pt[:])
            nc.sync.dma_start(n_d[:], nst[:])

    nc.compile()
    return nc


def _build_b():
    nc = bacc.Bacc("TRN2", target_bir_lowering=False, debug=False, num_devices=D)
    xpk_d = nc.dram_tensor("xpk", [128, KT, 1536], FP8, kind="ExternalInput").ap()
    wav_d = nc.dram_tensor("wav", [128, MT], F32, kind="ExternalInput").ap()
    # [0,0:NC]=wb_own, [0,NC:2NC]=nb_own, [0,2NC:2NC+MR]=na_own
    fv_d = nc.dram_tensor("fv", [1, 2 * NC + MR], F32, kind="ExternalInput").ap()
    out_d = nc.dram_tensor("scores", [MR, NC], BF16, kind="ExternalOutput").ap()

    with tile.TileContext(nc) as tc:
        with (
            tc.tile_pool(name="xp", bufs=1) as xp,
            tc.tile_pool(name="small", bufs=1) as small,
            tc.tile_pool(name="epi", bufs=1) as epip,
            tc.tile_pool(name="outp", bufs=2) as outp,
            tc.tile_pool(name="psm", bufs=1, space="PSUM") as psm,
        ):
            # x groups get the sync ring immediately (4x 1.57MB: fewer
            # per-DMA gaps); epilogue vectors ride the gpsimd ring
            BXG, BN = 8, 4
            xg = []
            for i in range(BN):
                x_t = xp.tile([128, BXG, 1536], FP8, tag=f"xg{i}")
                nc.sync.dma_start(x_t[:], xpk_d[:, i * BXG:(i + 1) * BXG, :])
                xg.append(x_t)
            wav = small.tile([128, MT], F32)
            nc.gpsimd.dma_start(wav[:], wav_d[:])
            wb_bc = epip.tile([128, NC], F32)
            nc.gpsimd.dma_start(wb_bc[:],
                                fv_d[0:1, 0:NC].to_broadcast([128, NC]))
            vrow = small.tile([1, NC + MR], F32)
            nc.gpsimd.dma_start(vrow[:], fv_d[0:1, NC:2 * NC + MR])
            nbna = small.tile([1, NC + MR], BF16)
            nc.vector.tensor_copy(nbna[:], vrow[:])
            ones_c = small.tile([1, 128], BF16)
            nc.vector.memset(ones_c[:], 1.0)
            ones_r = small.tile([1, 512], BF16)
            nc.vector.memset(ones_r[:], 1.0)
            # PE warmup: keep HAM busy from t~1us so the real matmul stream
            # runs at 2.4GHz from its first instruction
            wrm = small.tile([128, 2, 640], FP8)
            nc.vector.memset(wrm[:], 0.0)

            # w2[m] = (wa[i] + wb[j])^2 on ScalarE (overlaps x DMA)
            w2 = []
            for m in range(MT):
                w2m = epip.tile([128, NC], F32, name=f"w2_{m}", tag=f"w2{m}")
                nc.scalar.activation(
                    w2m[:], wb_bc[:], mybir.ActivationFunctionType.Square,
                    bias=wav[:, m:m + 1], scale=1.0)
                w2.append(w2m)

            ps = [psm.tile([128, NJ, 512], F32, name=f"ps{m}", tag=f"ps{m}")
                  for m in range(MT)]
            for _ in range(12):
                nc.tensor.matmul(
                    ps[0][:, 0, :], wrm[:, :, 0:128], wrm[:, :, 128:640],
                    start=True, stop=True, skip_group_check=True,
                    perf_mode=mybir.MatmulPerfMode.DoubleRow)
            # hybrid order: k-outer over chunks 0..2 (PE chases arrivals at
            # full density), then chunk 3 per-m with staggered stop so each
            # m-tile's epilogue overlaps the remaining matmuls
            for i in range(BN - 1):
                for s2 in range(BXG // 2):
                    first = (i == 0 and s2 == 0)
                    for m in range(MT):
                        for nj in range(NJ):
                            nc.tensor.matmul(
                                ps[m][:, nj, :],
                                xg[i][:, 2 * s2:2 * s2 + 2,
                                      m * 128:(m + 1) * 128],
                                xg[i][:, 2 * s2:2 * s2 + 2,
                                      MR + nj * 512:MR + (nj + 1) * 512],
                                start=first, stop=False,
                                perf_mode=mybir.MatmulPerfMode.DoubleRow)
            for m in range(MT):
                for s2 in range(BXG // 2):
                    for nj in range(NJ):
                        nc.tensor.matmul(
                            ps[m][:, nj, :],
                            xg[BN - 1][:, 2 * s2:2 * s2 + 2,
                                       m * 128:(m + 1) * 128],
                            xg[BN - 1][:, 2 * s2:2 * s2 + 2,
                                       MR + nj * 512:MR + (nj + 1) * 512],
                            start=False, stop=False,
                            perf_mode=mybir.MatmulPerfMode.DoubleRow)
                # fold na[i] / nb[j] into psum via K=1 matmuls
                for nj in range(NJ):
                    nc.tensor.matmul(
                        ps[m][:, nj, :],
                        nbna[0:1, NC + m * 128:NC + (m + 1) * 128],
                        ones_r[:],
                        start=False, stop=False, skip_group_check=True)
                    nc.tensor.matmul(
                        ps[m][:, nj, :],
                        ones_c[0:1, 0:128],
                        nbna[0:1, nj * 512:(nj + 1) * 512],
                        start=False, stop=True, skip_group_check=True)
                # epilogue: Scalar relu (psum->sbuf), DVE mul by w2, store
                pflat = ps[m].rearrange("p a b -> p (a b)")
                rl = outp.tile([128, NC], F32, tag="rl")
                nc.scalar.activation(rl[:], pflat,
                                     mybir.ActivationFunctionType.Relu)
                ot = outp.tile([128, NC], BF16, tag="ot")
                nc.vector.tensor_mul(ot[:], rl[:], w2[m][:])
                nc.sync.dma_start(out_d[m * 128:(m + 1) * 128, :], ot[:])

    nc.compile()
    return nc


def _p_major(a, np_dtype):
    n = a.shape[0] // 128
    return np.ascontiguousarray(
        a.reshape(n, 128, a.shape[1]).transpose(1, 0, 2).astype(np_dtype))


def kernel_v3(x, fc_weight, _trace=False):
    x = np.asarray(x, dtype=np.float32)
    fc = np.asarray(fc_weight, dtype=np.float32)
    xf = x.reshape(T, C)
    xa = np.ascontiguousarray(xf[:, 0::2])
    xb = np.ascontiguousarray(xf[:, 1::2])
    fc_r = np.concatenate([fc[:, 0::2], fc[:, 1::2]], axis=1)  # [O, even|odd]

    if "a" not in _cache:
        _cache["a"] = _build_a()
    if "b" not in _cache:
        _cache["b"] = _build_b()

    # ---- launch A ----
    in_a = []
    for d in range(D):
        xs = np.concatenate(
            [xa[512 * d:512 * (d + 1), :], xb[512 * d:512 * (d + 1), :]],
            axis=1)                                   # [512, 4096] = [xa|xb]
        in_a.append({
            "fcr": _p_major(fc_r[1536 * d:1536 * (d + 1), :], NP_BF16),
            "xsq": _p_major(xs, NP_FP8),
        })
    res_a = run_bass_kernel_spmd(_cache["a"], in_a, core_ids=list(range(D)),
                                 trace=_trace)
    t_a = res_a.exec_time_ns

    wsum = np.sum([res_a.results[d]["wpart"][0] for d in range(D)], axis=0,
                  dtype=np.float32)                   # [4096] = [wa|wb]
    nn = np.sum([res_a.results[d]["npart"][0] for d in range(D)], axis=0,
                dtype=np.float32)                     # [4096] = [na|nb]
    wa, wb = wsum[:CA], wsum[CA:]
    na, nb = nn[:CA], nn[CA:]

    # ---- launch B ----
    xa2 = xa * -2.0
    in_b = []
    pidx = (np.arange(128)[:, None] + 128 * np.arange(MT)[None, :])  # [128,MT]
    for d in range(D):
        r, c = d // 2, d % 2
        xa_p = _p_major(xa2[:, 512 * r:512 * (r + 1)], NP_FP8)
        xb_p = _p_major(xb[:, NC * c:NC * (c + 1)], NP_FP8)
        xpk = np.ascontiguousarray(np.concatenate([xa_p, xb_p], axis=2))
        in_b.append({
            "xpk": xpk,
            "wav": np.ascontiguousarray(wa[512 * r + pidx]).astype(np.float32),
            "fv": np.concatenate([wb[NC * c:NC * (c + 1)],
                                  nb[NC * c:NC * (c + 1)],
                                  na[512 * r:512 * (r + 1)]]).reshape(1, -1)
                    .astype(np.float32),
        })
    res_b = run_bass_kernel_spmd(_cache["b"], in_b, core_ids=list(range(D)),
                                 trace=_trace)
    t_b = res_b.exec_time_ns

    out = np.empty((CA, CA), dtype=np.float32)
    for d in range(D):
        r, c = d // 2, d % 2
        out[512 * r:512 * (r + 1), NC * c:NC * (c + 1)] = (
            res_b.results[d]["scores"].astype(np.float32))
    if _trace:
        kernel_v3.last_times = (t_a, t_b)
    return out


def kernel(x, fc_weight):
    """Graded entrypoint: full inputs in, full [2048, 2048] scores out."""
    return kernel_v3(x, fc_weight)
